# revision 10
# baseline (speedup 1.0000x reference)
"""Invertible 1x1 conv (Glow-style) on 8 Trainium2 NeuronCores.

z_out[b,o,h,w] = sum_c W[o,c] z[b,c,h,w],  W = Q @ R with
Q = prod_i (I - 2 v_i v_i^T / (v_i^T v_i)) (96 Householder reflections) and
R = triu(U,1) + diag(S).  logdet_out = logdet + sum(log|S|)*H*W.

Strategy: data-parallel over batch (2 images per core).  Each core
redundantly builds W^T on-device using the compact-WY representation:
    Q = I - V T V^T,  T^{-1} = striu(G) + diag(diag(G)/2),  G = V^T V
and a log-depth Neumann product for the triangular inverse
    (I+N)^{-1} = (I-N)(I+N^2)(I+N^4)...(I+N^64)   (N strictly upper, N^96=0)
which replaces the 96-step sequential scan with ~24 parallel 96x96 matmuls
that hide under the first z-tile DMA.  The bulk work is a [96x96] @ [96x16384]
channel-mixing matmul per image, streamed through the PE in N=512 slices,
memory-bound at ~25 MB of HBM traffic per core.
"""

import os
import sys

import numpy as np

for _p in ("/opt/trn_rl_repo",):
    if os.path.isdir(_p) and _p not in sys.path:
        sys.path.insert(0, _p)

B, C, H, W = 16, 96, 128, 128
N_CORES = 8
B_SH = B // N_CORES          # batch images per core
HW = H * W                   # 16384 pixels
N_TILE = 4096                # z tile free size (columns) per DMA
MM_N = 512                   # fp32 matmul free-dim limit (one PSUM bank)
ZBUFS_IN = 4
ZBUFS_OUT = 4
NUM_DEV_BUILD = 8

_CACHE = {}


def _build_program(n_tile=None, bufs_in=None, bufs_out=None, num_devices=None,
                   k_passes=1, tile_plan=None, timing_io=False):
    n_tile = n_tile or N_TILE
    bufs_in = bufs_in or ZBUFS_IN
    bufs_out = bufs_out or ZBUFS_OUT
    num_devices = num_devices or NUM_DEV_BUILD
    # per-image tile plan: ramp in/out with small tiles to shrink the
    # pipeline head (first matmul waits on first load) and tail (last store)
    if tile_plan is None:
        if n_tile == 4096:
            tile_plan = [1024, 3072, 4096, 4096, 4096]
        else:
            tile_plan = [n_tile] * (HW // n_tile)
    assert sum(tile_plan) == HW
    slot = max(tile_plan)
    plan = []          # (image, col offset, size) across the core's images
    for b in range(B_SH):
        off = 0
        sizes = tile_plan if b == 0 else list(reversed(tile_plan))
        for s in sizes:
            plan.append((b, off, s))
            off += s
    import concourse.bacc as bacc
    import concourse.bass as bass
    import concourse.mybir as mybir
    import concourse.tile as tile

    f32 = mybir.dt.float32
    nc = bacc.Bacc(
        "TRN2", target_bir_lowering=False, debug=False, num_devices=num_devices
    )

    z_shape = (B_SH, C, 1, 64) if timing_io else (B_SH, C, H, W)
    z_in = nc.dram_tensor("z", z_shape, f32, kind="ExternalInput").ap()
    ld_in = nc.dram_tensor("logdet", (B_SH,), f32, kind="ExternalInput").ap()
    s_in = nc.dram_tensor("S", (C,), f32, kind="ExternalInput").ap()
    u_in = nc.dram_tensor("U", (C, C), f32, kind="ExternalInput").ap()
    v_in = nc.dram_tensor("v", (C, C, 1), f32, kind="ExternalInput").ap()
    z_out = nc.dram_tensor("z_out", z_shape, f32, kind="ExternalOutput").ap()
    ld_out = nc.dram_tensor("logdet_out", (B_SH,), f32, kind="ExternalOutput").ap()

    z3 = z_in.rearrange("b c h w -> b c (h w)")      # [B_SH, C, HW]
    zo3 = z_out.rearrange("b c h w -> b c (h w)")

    # bounce buffers for multi-pass timing builds (k_passes > 1)
    bounce = []
    if k_passes > 1 or timing_io:
        for i in range(2):
            t = nc.dram_tensor("bounce%d" % i, (B_SH, C, HW), f32, kind="Internal").ap()
            bounce.append(t)

    with tile.TileContext(nc) as tc:
        with tc.tile_pool(name="consts", bufs=1) as cp, \
             tc.tile_pool(name="zin", bufs=bufs_in) as zin_p, \
             tc.tile_pool(name="zoutp", bufs=bufs_out) as zout_p, \
             tc.tile_pool(name="psA", bufs=2, space="PSUM") as psa_p, \
             tc.tile_pool(name="psB", bufs=6, space="PSUM") as psb_p:

            def ctile(tag):
                return cp.tile([C, C], f32, tag=tag, name=tag)

            def pa():
                return psa_p.tile([C, C], f32, tag="pa", name="pa")

            # ---- small input loads (issue first: tiny, feed phase A) ----
            vt_s = ctile("vt")                       # [i, d] rows are v_i
            nc.sync.dma_start(out=vt_s[:], in_=v_in.rearrange("i d one -> i (d one)"))
            u_s = ctile("u")
            nc.sync.dma_start(out=u_s[:], in_=u_in[:])
            s_row = cp.tile([1, C], f32, tag="srow", name="srow")
            nc.sync.dma_start(out=s_row[:], in_=s_in.unsqueeze(0))
            s_col = cp.tile([C, 1], f32, tag="scol", name="scol")
            nc.sync.dma_start(out=s_col[:], in_=s_in.unsqueeze(1))
            ld_row = cp.tile([1, B_SH], f32, tag="ldrow", name="ldrow")
            nc.sync.dma_start(out=ld_row[:], in_=ld_in.unsqueeze(0))

            # ---- bulk z loads (no deps; stream in behind the small loads) ----
            zts = []
            if not timing_io:
                for b, off, sz in plan:
                    zt = zin_p.tile([C, slot], f32, tag="zt", name="zt")
                    nc.sync.dma_start(out=zt[:, :sz], in_=z3[b, :, off:off + sz])
                    zts.append(zt)

            # ---- constants: ones + identity (gpsimd; idle otherwise) ----
            ones_s = ctile("ones")
            nc.gpsimd.memset(ones_s[:], 1.0)
            ident_s = ctile("ident")
            # keep where f - p == 0 -> identity
            nc.gpsimd.affine_select(
                out=ident_s[:], in_=ones_s[:], pattern=[[1, C]],
                compare_op=mybir.AluOpType.is_equal, fill=0.0,
                base=0, channel_multiplier=-1,
            )

            # ---- phase A: build W^T [c_in, c_out] ----
            # V = VT^T via PE transpose
            ps = pa()
            nc.tensor.transpose(ps[:], vt_s[:], ident_s[:])
            v_s = ctile("v")                         # [d, i]
            nc.vector.tensor_copy(v_s[:], ps[:])

            # G = V^T V  -> [i, j]
            ps = pa()
            nc.tensor.matmul(ps[:], lhsT=v_s[:], rhs=v_s[:])
            g_s = ctile("g")
            nc.vector.tensor_copy(g_s[:], ps[:])

            # diag(G) -> [C,1]; D = diag/2; recipD = 2/diag
            gmask = ctile("gmask")
            nc.vector.tensor_mul(gmask[:], g_s[:], ident_s[:])
            g_col = cp.tile([C, 1], f32, tag="gcol", name="gcol")
            nc.vector.tensor_reduce(
                g_col[:], gmask[:], axis=mybir.AxisListType.X, op=mybir.AluOpType.add
            )
            d_col = cp.tile([C, 1], f32, tag="dcol", name="dcol")
            nc.vector.tensor_scalar_mul(d_col[:], g_col[:], 0.5)
            rd_col = cp.tile([C, 1], f32, tag="rdcol", name="rdcol")
            nc.vector.reciprocal(rd_col[:], d_col[:])

            # N = striu(G) row-scaled by recipD  (strictly upper)
            g_up = ctile("gup")
            nc.gpsimd.affine_select(
                out=g_up[:], in_=g_s[:], pattern=[[1, C]],
                compare_op=mybir.AluOpType.is_ge, fill=0.0,
                base=-1, channel_multiplier=-1,
            )
            n_s = ctile("n")
            nc.vector.tensor_scalar_mul(n_s[:], g_up[:], rd_col[:])

            # NT = N^T via PE
            ps = pa()
            nc.tensor.transpose(ps[:], n_s[:], ident_s[:])
            nt_s = ctile("nt")
            nc.vector.tensor_copy(nt_s[:], ps[:])

            # squarings: P_{2k} = P_k @ P_k, keeping both orientations.
            # untransposed powers are the lhsT operands of the chain below.
            pows = []                                # untransposed P2..P64
            cur, curt = n_s, nt_s
            for k in range(6):
                ps = pa()
                nc.tensor.matmul(ps[:], lhsT=curt[:], rhs=cur[:])      # P @ P
                p_s = ctile(f"p{k}")
                nc.vector.tensor_copy(p_s[:], ps[:])
                if k < 5:
                    ps = pa()
                    nc.tensor.matmul(ps[:], lhsT=cur[:], rhs=curt[:])  # (P@P)^T
                    pt_s = ctile(f"pt{k}")
                    nc.vector.tensor_copy(pt_s[:], ps[:])
                else:
                    pt_s = None
                pows.append(p_s)
                cur, curt = p_s, pt_s

            # chain: Y = (I+P64^T)...(I+P2^T)(I-N^T) V^T ; E0 = D^{-1} Y
            ps = pa()
            nc.tensor.matmul(ps[:], lhsT=n_s[:], rhs=vt_s[:])          # N^T Y0
            y_s = ctile("y0")
            nc.vector.tensor_sub(y_s[:], vt_s[:], ps[:])
            for k, p_s in enumerate(pows):
                ps = pa()
                nc.tensor.matmul(ps[:], lhsT=p_s[:], rhs=y_s[:])       # P^T Y
                y2_s = ctile(f"y{k + 1}")
                nc.vector.tensor_add(y2_s[:], y_s[:], ps[:])
                y_s = y2_s
            e0_s = ctile("e0")
            nc.vector.tensor_scalar_mul(e0_s[:], y_s[:], rd_col[:])

            # R = striu(U) + diag(S);  R^T via PE
            u_up = ctile("uup")
            nc.gpsimd.affine_select(
                out=u_up[:], in_=u_s[:], pattern=[[1, C]],
                compare_op=mybir.AluOpType.is_ge, fill=0.0,
                base=-1, channel_multiplier=-1,
            )
            sdiag = ctile("sdiag")
            nc.vector.tensor_scalar_mul(sdiag[:], ident_s[:], s_col[:])
            r_s = ctile("r")
            nc.vector.tensor_add(r_s[:], u_up[:], sdiag[:])
            ps = pa()
            nc.tensor.transpose(ps[:], r_s[:], ident_s[:])
            rt_s = ctile("rt")
            nc.vector.tensor_copy(rt_s[:], ps[:])

            # C0 = V^T R  -> [i, c]
            ps = pa()
            nc.tensor.matmul(ps[:], lhsT=v_s[:], rhs=r_s[:])
            c0_s = ctile("c0")
            nc.vector.tensor_copy(c0_s[:], ps[:])

            # W^T = R^T - C0^T E0  -> [c_in, c_out]
            ps = pa()
            nc.tensor.matmul(ps[:], lhsT=c0_s[:], rhs=e0_s[:])
            wt_s = ctile("wt")
            nc.vector.tensor_sub(wt_s[:], rt_s[:], ps[:])

            # ---- logdet (scalar engine) ----
            abs_s = cp.tile([1, C], f32, tag="sabs", name="sabs")
            nc.scalar.activation(abs_s[:], s_row[:], mybir.ActivationFunctionType.Abs)
            ln_s = cp.tile([1, C], f32, tag="sln", name="sln")
            ksum = cp.tile([1, 1], f32, tag="ksum", name="ksum")
            nc.scalar.activation(
                ln_s[:], abs_s[:], mybir.ActivationFunctionType.Ln,
                accum_out=ksum[:],
            )
            khw = cp.tile([1, 1], f32, tag="khw", name="khw")
            nc.scalar.activation(
                khw[:], ksum[:], mybir.ActivationFunctionType.Copy, scale=float(HW)
            )
            ldo_row = cp.tile([1, B_SH], f32, tag="ldorow", name="ldorow")
            nc.vector.tensor_scalar_add(ldo_row[:], ld_row[:], khw[:])
            nc.scalar.dma_start(out=ld_out.unsqueeze(0), in_=ldo_row[:])

            # ---- phase B: z_out tiles = W @ z tiles ----
            def pass_body(src3, dst3, in_tiles=None):
                for t, (b, off, sz) in enumerate(plan):
                    if in_tiles is not None:
                        zt = in_tiles[t]
                    else:
                        zt = zin_p.tile([C, slot], f32, tag="zt", name="zt")
                        nc.sync.dma_start(out=zt[:, :sz], in_=src3[b, :, off:off + sz])
                    ot = zout_p.tile([C, slot], f32, tag="ot", name="ot")
                    for j in range(sz // MM_N):
                        psb = psb_p.tile([C, MM_N], f32, tag="mm", name="mm")
                        nc.tensor.matmul(
                            psb[:], lhsT=wt_s[:], rhs=zt[:, j * MM_N:(j + 1) * MM_N]
                        )
                        nc.vector.tensor_copy(ot[:, j * MM_N:(j + 1) * MM_N], psb[:])
                    nc.scalar.dma_start(out=dst3[b, :, off:off + sz], in_=ot[:, :sz])

            if timing_io:
                # tiny I/O timing mode: all passes loop over bounce buffers
                # (uninitialized data -- timing only); tiny z/z_out keep the
                # external tensors bound without bulk transfers.
                tiny = cp.tile([C, B_SH * 64], f32, tag="tiny", name="tiny")
                for i in range(B_SH):
                    nc.sync.dma_start(out=tiny[:, i * 64:(i + 1) * 64], in_=z3[i])
                for p in range(k_passes):
                    pass_body(bounce[p % 2], bounce[(p + 1) % 2])
                for i in range(B_SH):
                    nc.scalar.dma_start(out=zo3[i], in_=tiny[:, i * 64:(i + 1) * 64])
            elif k_passes == 1:
                pass_body(z3, zo3, in_tiles=zts)
            else:
                pass_body(z3, bounce[0], in_tiles=zts)
                for p in range(1, k_passes - 1):
                    pass_body(bounce[(p - 1) % 2], bounce[p % 2])
                pass_body(bounce[(k_passes - 2) % 2], zo3)

    nc.compile()
    return nc


def _get_program():
    if "nc" not in _CACHE:
        _CACHE["nc"] = _build_program()
    return _CACHE["nc"]


def kernel(z, logdet, S, U, v):
    from concourse.bass_utils import run_bass_kernel_spmd

    nc = _get_program()
    z = np.ascontiguousarray(z, dtype=np.float32)
    logdet = np.ascontiguousarray(logdet, dtype=np.float32)
    S = np.ascontiguousarray(S, dtype=np.float32)
    U = np.ascontiguousarray(U, dtype=np.float32)
    v = np.ascontiguousarray(v, dtype=np.float32)

    in_maps = [
        {
            "z": z[i * B_SH:(i + 1) * B_SH],
            "logdet": logdet[i * B_SH:(i + 1) * B_SH],
            "S": S,
            "U": U,
            "v": v,
        }
        for i in range(N_CORES)
    ]
    res = run_bass_kernel_spmd(nc, in_maps, list(range(N_CORES)))
    z_full = np.concatenate([r["z_out"] for r in res.results], axis=0)
    ld_full = np.concatenate([r["logdet_out"] for r in res.results], axis=0)
    return z_full, ld_full


if __name__ == "__main__":
    rng = np.random.default_rng(0)
    inputs = {
        "z": rng.standard_normal((B, C, H, W), dtype=np.float32),
        "logdet": np.zeros((B,), np.float32),
        "S": rng.standard_normal((C,), dtype=np.float32),
        "U": rng.standard_normal((C, C), dtype=np.float32),
        "v": rng.standard_normal((C, C, 1), dtype=np.float32),
    }
    zo, ldo = kernel(**inputs)
    print("z_out", zo.shape, zo.dtype, "logdet_out", ldo.shape, ldo.dtype)


# revision 11
# speedup vs baseline: 1.0037x; 1.0037x over previous
"""Invertible 1x1 conv (Glow-style) on 8 Trainium2 NeuronCores.

z_out[b,o,h,w] = sum_c W[o,c] z[b,c,h,w],  W = Q @ R with
Q = prod_i (I - 2 v_i v_i^T / (v_i^T v_i)) (96 Householder reflections) and
R = triu(U,1) + diag(S).  logdet_out = logdet + sum(log|S|)*H*W.

Strategy: data-parallel over batch (2 images per core).  Each core
redundantly builds W^T on-device using the compact-WY representation:
    Q = I - V T V^T,  T^{-1} = striu(G) + diag(diag(G)/2),  G = V^T V
and a log-depth Neumann product for the triangular inverse
    (I+N)^{-1} = (I-N)(I+N^2)(I+N^4)...(I+N^64)   (N strictly upper, N^96=0)
which replaces the 96-step sequential scan with ~24 parallel 96x96 matmuls
that hide under the first z-tile DMA.  The bulk work is a [96x96] @ [96x16384]
channel-mixing matmul per image, streamed through the PE in N=512 slices,
memory-bound at ~25 MB of HBM traffic per core.
"""

import os
import sys

import numpy as np

for _p in ("/opt/trn_rl_repo",):
    if os.path.isdir(_p) and _p not in sys.path:
        sys.path.insert(0, _p)

B, C, H, W = 16, 96, 128, 128
N_CORES = 8
B_SH = B // N_CORES          # batch images per core
HW = H * W                   # 16384 pixels
N_TILE = 4096                # z tile free size (columns) per DMA
MM_N = 512                   # fp32 matmul free-dim limit (one PSUM bank)
ZBUFS_IN = 5
ZBUFS_OUT = 4
NUM_DEV_BUILD = 8

_CACHE = {}


def _build_program(n_tile=None, bufs_in=None, bufs_out=None, num_devices=None,
                   k_passes=1, tile_plan=None, timing_io=False):
    n_tile = n_tile or N_TILE
    bufs_in = bufs_in or ZBUFS_IN
    bufs_out = bufs_out or ZBUFS_OUT
    num_devices = num_devices or NUM_DEV_BUILD
    # per-image tile plan: ramp in/out with small tiles to shrink the
    # pipeline head (first matmul waits on first load) and tail (last store)
    if tile_plan is None:
        if n_tile == 4096:
            tile_plan = [1024, 3072, 4096, 4096, 4096]
        else:
            tile_plan = [n_tile] * (HW // n_tile)
    assert sum(tile_plan) == HW
    slot = max(tile_plan)
    plan = []          # (image, col offset, size) across the core's images
    for b in range(B_SH):
        off = 0
        sizes = tile_plan if b == 0 else list(reversed(tile_plan))
        for s in sizes:
            plan.append((b, off, s))
            off += s
    import concourse.bacc as bacc
    import concourse.bass as bass
    import concourse.mybir as mybir
    import concourse.tile as tile

    f32 = mybir.dt.float32
    nc = bacc.Bacc(
        "TRN2", target_bir_lowering=False, debug=False, num_devices=num_devices
    )

    z_shape = (B_SH, C, 1, 64) if timing_io else (B_SH, C, H, W)
    z_in = nc.dram_tensor("z", z_shape, f32, kind="ExternalInput").ap()
    ld_in = nc.dram_tensor("logdet", (B_SH,), f32, kind="ExternalInput").ap()
    s_in = nc.dram_tensor("S", (C,), f32, kind="ExternalInput").ap()
    u_in = nc.dram_tensor("U", (C, C), f32, kind="ExternalInput").ap()
    v_in = nc.dram_tensor("v", (C, C, 1), f32, kind="ExternalInput").ap()
    z_out = nc.dram_tensor("z_out", z_shape, f32, kind="ExternalOutput").ap()
    ld_out = nc.dram_tensor("logdet_out", (B_SH,), f32, kind="ExternalOutput").ap()

    z3 = z_in.rearrange("b c h w -> b c (h w)")      # [B_SH, C, HW]
    zo3 = z_out.rearrange("b c h w -> b c (h w)")

    # bounce buffers for multi-pass timing builds (k_passes > 1)
    bounce = []
    if k_passes > 1 or timing_io:
        for i in range(2):
            t = nc.dram_tensor("bounce%d" % i, (B_SH, C, HW), f32, kind="Internal").ap()
            bounce.append(t)

    with tile.TileContext(nc) as tc:
        with tc.tile_pool(name="consts", bufs=1) as cp, \
             tc.tile_pool(name="zin", bufs=bufs_in) as zin_p, \
             tc.tile_pool(name="zoutp", bufs=bufs_out) as zout_p, \
             tc.tile_pool(name="psA", bufs=2, space="PSUM") as psa_p, \
             tc.tile_pool(name="psB", bufs=6, space="PSUM") as psb_p:

            def ctile(tag):
                return cp.tile([C, C], f32, tag=tag, name=tag)

            def pa():
                return psa_p.tile([C, C], f32, tag="pa", name="pa")

            # ---- small input loads (issue first: tiny, feed phase A) ----
            vt_s = ctile("vt")                       # [i, d] rows are v_i
            nc.sync.dma_start(out=vt_s[:], in_=v_in.rearrange("i d one -> i (d one)"))
            u_s = ctile("u")
            nc.sync.dma_start(out=u_s[:], in_=u_in[:])
            s_row = cp.tile([1, C], f32, tag="srow", name="srow")
            nc.sync.dma_start(out=s_row[:], in_=s_in.unsqueeze(0))
            s_col = cp.tile([C, 1], f32, tag="scol", name="scol")
            nc.sync.dma_start(out=s_col[:], in_=s_in.unsqueeze(1))
            ld_row = cp.tile([1, B_SH], f32, tag="ldrow", name="ldrow")
            nc.sync.dma_start(out=ld_row[:], in_=ld_in.unsqueeze(0))

            # ---- bulk z loads (no deps; stream in behind the small loads) ----
            zts = []
            if not timing_io:
                for b, off, sz in plan:
                    zt = zin_p.tile([C, slot], f32, tag="zt", name="zt")
                    nc.sync.dma_start(out=zt[:, :sz], in_=z3[b, :, off:off + sz])
                    zts.append(zt)

            # ---- constants: ones + identity (gpsimd; idle otherwise) ----
            ones_s = ctile("ones")
            nc.gpsimd.memset(ones_s[:], 1.0)
            ident_s = ctile("ident")
            # keep where f - p == 0 -> identity
            nc.gpsimd.affine_select(
                out=ident_s[:], in_=ones_s[:], pattern=[[1, C]],
                compare_op=mybir.AluOpType.is_equal, fill=0.0,
                base=0, channel_multiplier=-1,
            )

            # ---- phase A: build W^T [c_in, c_out] ----
            # V = VT^T via PE transpose
            ps = pa()
            nc.tensor.transpose(ps[:], vt_s[:], ident_s[:])
            v_s = ctile("v")                         # [d, i]
            nc.vector.tensor_copy(v_s[:], ps[:])

            # G = V^T V  -> [i, j]
            ps = pa()
            nc.tensor.matmul(ps[:], lhsT=v_s[:], rhs=v_s[:])
            g_s = ctile("g")
            nc.vector.tensor_copy(g_s[:], ps[:])

            # diag(G) -> [C,1]; D = diag/2; recipD = 2/diag
            gmask = ctile("gmask")
            nc.vector.tensor_mul(gmask[:], g_s[:], ident_s[:])
            g_col = cp.tile([C, 1], f32, tag="gcol", name="gcol")
            nc.vector.tensor_reduce(
                g_col[:], gmask[:], axis=mybir.AxisListType.X, op=mybir.AluOpType.add
            )
            d_col = cp.tile([C, 1], f32, tag="dcol", name="dcol")
            nc.vector.tensor_scalar_mul(d_col[:], g_col[:], 0.5)
            rd_col = cp.tile([C, 1], f32, tag="rdcol", name="rdcol")
            nc.vector.reciprocal(rd_col[:], d_col[:])

            # N = striu(G) row-scaled by recipD  (strictly upper)
            g_up = ctile("gup")
            nc.gpsimd.affine_select(
                out=g_up[:], in_=g_s[:], pattern=[[1, C]],
                compare_op=mybir.AluOpType.is_ge, fill=0.0,
                base=-1, channel_multiplier=-1,
            )
            n_s = ctile("n")
            nc.vector.tensor_scalar_mul(n_s[:], g_up[:], rd_col[:])

            # NT = N^T via PE
            ps = pa()
            nc.tensor.transpose(ps[:], n_s[:], ident_s[:])
            nt_s = ctile("nt")
            nc.vector.tensor_copy(nt_s[:], ps[:])

            # squarings: P_{2k} = P_k @ P_k, keeping both orientations.
            # untransposed powers are the lhsT operands of the chain below.
            pows = []                                # untransposed P2..P64
            cur, curt = n_s, nt_s
            for k in range(6):
                ps = pa()
                nc.tensor.matmul(ps[:], lhsT=curt[:], rhs=cur[:])      # P @ P
                p_s = ctile(f"p{k}")
                nc.vector.tensor_copy(p_s[:], ps[:])
                if k < 5:
                    ps = pa()
                    nc.tensor.matmul(ps[:], lhsT=cur[:], rhs=curt[:])  # (P@P)^T
                    pt_s = ctile(f"pt{k}")
                    nc.vector.tensor_copy(pt_s[:], ps[:])
                else:
                    pt_s = None
                pows.append(p_s)
                cur, curt = p_s, pt_s

            # chain: Y = (I+P64^T)...(I+P2^T)(I-N^T) V^T ; E0 = D^{-1} Y
            ps = pa()
            nc.tensor.matmul(ps[:], lhsT=n_s[:], rhs=vt_s[:])          # N^T Y0
            y_s = ctile("y0")
            nc.vector.tensor_sub(y_s[:], vt_s[:], ps[:])
            for k, p_s in enumerate(pows):
                ps = pa()
                nc.tensor.matmul(ps[:], lhsT=p_s[:], rhs=y_s[:])       # P^T Y
                y2_s = ctile(f"y{k + 1}")
                nc.vector.tensor_add(y2_s[:], y_s[:], ps[:])
                y_s = y2_s
            e0_s = ctile("e0")
            nc.vector.tensor_scalar_mul(e0_s[:], y_s[:], rd_col[:])

            # R = striu(U) + diag(S);  R^T via PE
            u_up = ctile("uup")
            nc.gpsimd.affine_select(
                out=u_up[:], in_=u_s[:], pattern=[[1, C]],
                compare_op=mybir.AluOpType.is_ge, fill=0.0,
                base=-1, channel_multiplier=-1,
            )
            sdiag = ctile("sdiag")
            nc.vector.tensor_scalar_mul(sdiag[:], ident_s[:], s_col[:])
            r_s = ctile("r")
            nc.vector.tensor_add(r_s[:], u_up[:], sdiag[:])
            ps = pa()
            nc.tensor.transpose(ps[:], r_s[:], ident_s[:])
            rt_s = ctile("rt")
            nc.vector.tensor_copy(rt_s[:], ps[:])

            # C0 = V^T R  -> [i, c]
            ps = pa()
            nc.tensor.matmul(ps[:], lhsT=v_s[:], rhs=r_s[:])
            c0_s = ctile("c0")
            nc.vector.tensor_copy(c0_s[:], ps[:])

            # W^T = R^T - C0^T E0  -> [c_in, c_out]
            ps = pa()
            nc.tensor.matmul(ps[:], lhsT=c0_s[:], rhs=e0_s[:])
            wt_s = ctile("wt")
            nc.vector.tensor_sub(wt_s[:], rt_s[:], ps[:])

            # ---- logdet (scalar engine) ----
            abs_s = cp.tile([1, C], f32, tag="sabs", name="sabs")
            nc.scalar.activation(abs_s[:], s_row[:], mybir.ActivationFunctionType.Abs)
            ln_s = cp.tile([1, C], f32, tag="sln", name="sln")
            ksum = cp.tile([1, 1], f32, tag="ksum", name="ksum")
            nc.scalar.activation(
                ln_s[:], abs_s[:], mybir.ActivationFunctionType.Ln,
                accum_out=ksum[:],
            )
            khw = cp.tile([1, 1], f32, tag="khw", name="khw")
            nc.scalar.activation(
                khw[:], ksum[:], mybir.ActivationFunctionType.Copy, scale=float(HW)
            )
            ldo_row = cp.tile([1, B_SH], f32, tag="ldorow", name="ldorow")
            nc.vector.tensor_scalar_add(ldo_row[:], ld_row[:], khw[:])
            nc.scalar.dma_start(out=ld_out.unsqueeze(0), in_=ldo_row[:])

            # ---- phase B: z_out tiles = W @ z tiles ----
            def pass_body(src3, dst3, in_tiles=None):
                for t, (b, off, sz) in enumerate(plan):
                    if in_tiles is not None:
                        zt = in_tiles[t]
                    else:
                        zt = zin_p.tile([C, slot], f32, tag="zt", name="zt")
                        nc.sync.dma_start(out=zt[:, :sz], in_=src3[b, :, off:off + sz])
                    ot = zout_p.tile([C, slot], f32, tag="ot", name="ot")
                    for j in range(sz // MM_N):
                        psb = psb_p.tile([C, MM_N], f32, tag="mm", name="mm")
                        nc.tensor.matmul(
                            psb[:], lhsT=wt_s[:], rhs=zt[:, j * MM_N:(j + 1) * MM_N]
                        )
                        nc.vector.tensor_copy(ot[:, j * MM_N:(j + 1) * MM_N], psb[:])
                    nc.scalar.dma_start(out=dst3[b, :, off:off + sz], in_=ot[:, :sz])

            if timing_io:
                # tiny I/O timing mode: all passes loop over bounce buffers
                # (uninitialized data -- timing only); tiny z/z_out keep the
                # external tensors bound without bulk transfers.
                tiny = cp.tile([C, B_SH * 64], f32, tag="tiny", name="tiny")
                for i in range(B_SH):
                    nc.sync.dma_start(out=tiny[:, i * 64:(i + 1) * 64], in_=z3[i])
                for p in range(k_passes):
                    pass_body(bounce[p % 2], bounce[(p + 1) % 2])
                for i in range(B_SH):
                    nc.scalar.dma_start(out=zo3[i], in_=tiny[:, i * 64:(i + 1) * 64])
            elif k_passes == 1:
                pass_body(z3, zo3, in_tiles=zts)
            else:
                pass_body(z3, bounce[0], in_tiles=zts)
                for p in range(1, k_passes - 1):
                    pass_body(bounce[(p - 1) % 2], bounce[p % 2])
                pass_body(bounce[(k_passes - 2) % 2], zo3)

    nc.compile()
    return nc


def _get_program():
    if "nc" not in _CACHE:
        _CACHE["nc"] = _build_program()
    return _CACHE["nc"]


def kernel(z, logdet, S, U, v):
    from concourse.bass_utils import run_bass_kernel_spmd

    nc = _get_program()
    z = np.ascontiguousarray(z, dtype=np.float32)
    logdet = np.ascontiguousarray(logdet, dtype=np.float32)
    S = np.ascontiguousarray(S, dtype=np.float32)
    U = np.ascontiguousarray(U, dtype=np.float32)
    v = np.ascontiguousarray(v, dtype=np.float32)

    in_maps = [
        {
            "z": z[i * B_SH:(i + 1) * B_SH],
            "logdet": logdet[i * B_SH:(i + 1) * B_SH],
            "S": S,
            "U": U,
            "v": v,
        }
        for i in range(N_CORES)
    ]
    res = run_bass_kernel_spmd(nc, in_maps, list(range(N_CORES)))
    z_full = np.concatenate([r["z_out"] for r in res.results], axis=0)
    ld_full = np.concatenate([r["logdet_out"] for r in res.results], axis=0)
    return z_full, ld_full


if __name__ == "__main__":
    rng = np.random.default_rng(0)
    inputs = {
        "z": rng.standard_normal((B, C, H, W), dtype=np.float32),
        "logdet": np.zeros((B,), np.float32),
        "S": rng.standard_normal((C,), dtype=np.float32),
        "U": rng.standard_normal((C, C), dtype=np.float32),
        "v": rng.standard_normal((C, C, 1), dtype=np.float32),
    }
    zo, ldo = kernel(**inputs)
    print("z_out", zo.shape, zo.dtype, "logdet_out", ldo.shape, ldo.dtype)
